# revision 24
# baseline (speedup 1.0000x reference)
"""ARMA GNN (2-layer, 2-stack) on 8 Trainium2 NeuronCores.

Strategy (src-sharded graph parallelism):
  - Nodes are relabeled into variable-size target windows (<=128 nodes each),
    FFD-packed on the host so every window receives <= 256 edges from every
    source core -> every window needs exactly 2 gather groups on all cores.
  - Each core keeps its slice's features as a bf16 [SL+1, 128] DRAM table
    (last row = zeros for padding) and dma_gathers per-edge messages for
    edges whose SOURCE lives in its slice (int16 indices stay in range).
  - Per target window, a one-hot matmul (S^T @ M) aggregates messages into
    PSUM; 4 windows share one PSUM bank ([128,512] f32) and are flushed
    with a single copy + strided DMA into the partial buffer.
  - The schedule is j-major (window j of every core before window j+1); the
    partial buffer splits into NCHUNK window-ranges whose ReduceScatter(add)
    is issued mid-prop, and the post-RS per-window loop for chunk q is
    emitted interleaved into the following calls' gather stream so it
    overlaps the remainder of the prop.
  - Gathers run 32 groups (4096 indices) per dma_gather call to amortize
    SWDGE fixed cost and HWDGE issue cost; inter-prop loops are batched
    4 windows per iteration.
  - Degrees/normalization (deg^-1/2) are precomputed on the host (structural
    data, like the edge indices) and shipped as a per-core [128, WPC] input;
    source-side scale is folded into table rows, target-side applied to the
    aggregate.
"""

import os
import sys
import numpy as np

for _p in ("/root/.axon_site", "/root/.axon_site/_ro/trn_rl_repo",
           "/root/.axon_site/_ro/pypackages", "/opt/trn_rl_repo"):
    if os.path.isdir(_p) and _p not in sys.path:
        sys.path.append(_p)

import ml_dtypes

N = 100000
NC = 8
SL0 = 12544               # origin-slice size (maps node id -> source core)
# SL/WPC/NPAD/W/TABROWS are set by _prep once the variable-size window
# packing is known (WPC ~ 107, every window <= 256 edges per source core).
SL = None
NPAD = None
WPC = None
W = None
TABROWS = None
F_IN = 100
H = 64
C = 18
K = 2
F1 = K * H                # 128 (both stacks packed)
F2 = K * C                # 36
GROUPS_PER_CALL = 20      # 2560 idxs per dma_gather call
NCHUNK = 4                # ReduceScatter window-range chunks per prop
DELAY_CALLS = 35          # calls to wait after RS(q) before its loop iters
BF16 = ml_dtypes.bfloat16


def _install_ntff_hook():
    try:
        import types
        if 'antenv.axon_hooks' in sys.modules:
            return True
        from trn_agent_boot.trn_boot import _ntff_profile_via_ctypes
        hook = _ntff_profile_via_ctypes('/opt/axon/libaxon_pjrt.so')
        if hook is None:
            return False
        mod = types.ModuleType('antenv.axon_hooks')
        mod.get_axon_ntff_profile_hook = lambda: hook
        mod.set_axon_ntff_profile_hook = lambda h: None
        sys.modules['antenv.axon_hooks'] = mod
        import antenv
        antenv.axon_hooks = mod
        return True
    except Exception:
        return False


# ---------------------------------------------------------------- host prep

def _pack_windows(dd, cap=256, maxn=128):
    """FFD vector bin packing: per-source-core loads <= cap, <= maxn nodes.
    Returns bin_of, pos_of, num_bins."""
    n = dd.shape[0]
    order = np.argsort(-dd.max(axis=1), kind="stable")
    loads = np.zeros((0, NC), np.int64)
    counts = []
    bin_of = np.empty(n, np.int64)
    pos_of = np.empty(n, np.int64)
    for idx in order:
        dv = dd[idx]
        ok = np.nonzero(((loads + dv[None, :]) <= cap).all(axis=1))[0]
        placed = False
        for b in ok:
            if counts[b] < maxn:
                bin_of[idx] = b
                pos_of[idx] = counts[b]
                loads[b] += dv
                counts[b] += 1
                placed = True
                break
        if not placed:
            bin_of[idx] = len(counts)
            pos_of[idx] = 0
            loads = np.vstack([loads, dv[None, :]])
            counts.append(1)
    return bin_of, pos_of, len(counts)


def _prep(x, edge_index):
    global SL, NPAD, WPC, W, TABROWS
    src = np.asarray(edge_index[0], np.int64)
    tgt = np.asarray(edge_index[1], np.int64)
    E = src.shape[0]

    # source core of each edge: origin slices of SL0 nodes
    src_core = np.minimum(src // SL0, NC - 1)

    # per-target, per-source-core degree  [N, NC]
    deg_cd = np.zeros((N, NC), np.int32)
    np.add.at(deg_cd, (tgt, src_core), 1)

    # pack each target slice into variable-size windows (all <= 2 groups)
    packs = []
    nbins = 0
    for d in range(NC):
        lo, hi = d * SL0, min((d + 1) * SL0, N)
        bin_of, pos_of, nb = _pack_windows(deg_cd[lo:hi])
        packs.append((lo, hi, bin_of, pos_of))
        nbins = max(nbins, nb)
    WPC = nbins
    SL = WPC * 128
    NPAD = SL * NC
    W = WPC * NC
    TABROWS = SL + 1

    P = np.empty(N, np.int64)
    for d, (lo, hi, bin_of, pos_of) in enumerate(packs):
        P[np.arange(lo, hi)] = d * SL + bin_of * 128 + pos_of

    srcp = P[src]
    tgtp = P[tgt]

    # schedule order: j-major — window w = c*WPC + j runs at pos j*NC + c, so
    # window-range chunks complete in order and their ReduceScatter overlaps
    # the remaining chunks' compute.
    wids = np.arange(W)
    SPOS = (wids % WPC) * NC + (wids // WPC)       # window id -> sched pos
    ORDER_W = np.argsort(SPOS)                     # sched pos -> window id

    # per-core edge lists (by source core; P preserves the core)
    per_core = []
    e_cw = np.zeros((NC, W), np.int64)
    for c in range(NC):
        m = src_core == c
        s_loc = (srcp[m] - c * SL).astype(np.int64)
        t_glob = tgtp[m]
        w_of0 = t_glob // 128
        o = np.argsort(SPOS[w_of0], kind="stable")
        s_loc, t_glob = s_loc[o], t_glob[o]
        w_of = t_glob // 128
        np.add.at(e_cw[c], w_of, 1)
        per_core.append((s_loc, t_glob, w_of))

    Gw = np.maximum(np.ceil(e_cw.max(axis=0) / 128).astype(np.int64), 1)
    G_total = int(Gw.sum())
    Gw_s = Gw[ORDER_W]                             # groups per sched pos
    win_start_s = np.zeros(W + 1, np.int64)
    win_start_s[1:] = np.cumsum(Gw_s)

    # schedule (same on all cores): for group g -> (window, first, last)
    sched = []
    for s in range(W):
        w = int(ORDER_W[s])
        for jj in range(int(Gw_s[s])):
            sched.append((w, jj == 0, jj == int(Gw_s[s]) - 1))

    # per-core idx / tgtoff arrays
    idx_all = np.full((NC, G_total, 128), SL, np.int16)       # zero row pad
    tgo_all = np.full((NC, G_total, 128), -1.0, np.float32)   # dead pad
    for c in range(NC):
        s_loc, t_glob, w_of = per_core[c]
        sp = SPOS[w_of]                            # ascending
        within = np.zeros(len(s_loc), np.int64)
        start_idx = np.searchsorted(sp, np.arange(W), side="left")
        cnt = np.searchsorted(sp, np.arange(W), side="right") - start_idx
        for s in np.nonzero(cnt)[0]:
            a = start_idx[s]
            k = cnt[s]
            within[a:a + k] = np.arange(k)
        g_of = win_start_s[sp] + within // 128
        p_of = within % 128
        idx_all[c, g_of, p_of] = s_loc.astype(np.int16)
        tgo_all[c, g_of, p_of] = (t_glob - w_of * 128).astype(np.float32)

    # idx DRAM layout: [128 partitions, G_total*8] int16 (16-wrap, x8 replica)
    idx_dram = np.empty((NC, 128, G_total * 8), np.int16)
    for c in range(NC):
        lin = idx_all[c].reshape(-1)                     # [G*128]
        wrap = lin.reshape(-1, 16).T                     # [16, G*8]
        idx_dram[c] = np.tile(wrap, (8, 1))
    # tgtoff DRAM: [128, G_total] bf16 (column per group)
    tgo_dram = np.ascontiguousarray(
        tgo_all.transpose(0, 2, 1)).astype(BF16)         # [NC, 128, G]

    # x, transposed per slice, bf16, padded
    xpad = np.zeros((NPAD, F_IN), np.float32)
    xpad[P] = np.asarray(x, np.float32)
    xT = np.ascontiguousarray(
        xpad.reshape(NC, SL, F_IN).transpose(0, 2, 1)).astype(BF16)

    # symmetric-norm degree scale, computed on host (structural data, like P)
    deg = np.bincount(tgt, minlength=N).astype(np.float64)
    dis_host = np.where(deg > 0, 1.0 / np.sqrt(np.maximum(deg, 1.0)), 0.0)
    dis_pad = np.zeros(NPAD, np.float32)
    dis_pad[P] = dis_host.astype(np.float32)
    # per core: [128, WPC] (partition = node-in-window, col = window)
    dis_core = np.ascontiguousarray(
        dis_pad.reshape(NC, WPC, 128).transpose(0, 2, 1))

    return P, idx_dram, tgo_dram, xT, G_total, sched, dis_core


# ------------------------------------------------------------- bass program

def _build(G_total, sched, weights):
    from concourse import bacc, mybir
    from concourse.tile import TileContext
    import concourse.bass as bass

    nc = bacc.Bacc("TRN2", num_swdge_queues=4,
                   dynamic_dma_scratch_size=40960)
    dt = mybir.dt

    xT_p = nc.declare_dram_parameter("xT", [F_IN, SL], dt.bfloat16, isOutput=False)
    idx_p = nc.declare_dram_parameter("idx", [128, G_total * 8], dt.int16, isOutput=False)
    tgo_p = nc.declare_dram_parameter("tgo", [128, G_total], dt.bfloat16, isOutput=False)
    w1f_p = nc.declare_dram_parameter("w1f", [F_IN, 256], dt.bfloat16, isOutput=False)
    w1b_p = nc.declare_dram_parameter("w1b", [F1, F1], dt.bfloat16, isOutput=False)
    iw2_p = nc.declare_dram_parameter("iw2", [H, F2], dt.bfloat16, isOutput=False)
    rw2_p = nc.declare_dram_parameter("rw2", [H, F2], dt.bfloat16, isOutput=False)
    w2b_p = nc.declare_dram_parameter("w2b", [F2, F2], dt.bfloat16, isOutput=False)
    b1_p = nc.declare_dram_parameter("b1t", [128, F1], dt.float32, isOutput=False)
    b2_p = nc.declare_dram_parameter("b2t", [128, F2], dt.float32, isOutput=False)
    iota_p = nc.declare_dram_parameter("iota", [128, 128], dt.bfloat16, isOutput=False)
    eye_p = nc.declare_dram_parameter("eye", [128, 128], dt.bfloat16, isOutput=False)
    dis_p = nc.declare_dram_parameter("dis", [128, WPC], dt.float32, isOutput=False)
    out_p = nc.declare_dram_parameter("out", [SL, C], dt.float32, isOutput=True)

    NCALLS = (G_total + GROUPS_PER_CALL - 1) // GROUPS_PER_CALL

    # chunk boundaries in j (window-in-core index): big early chunks so
    # their ReduceScatter overlaps the prop, tiny last chunk so the
    # post-prop serial tail (RS + loop of the last chunk) stays short.
    fracs = [0.30, 0.60, 0.90]
    bounds = [0] + [round(f * WPC) for f in fracs] + [WPC]
    hq = [bounds[q + 1] - bounds[q] for q in range(NCHUNK)]
    assert all(h > 0 for h in hq)

    with TileContext(nc) as tc:
        with (
            tc.tile_pool(name="dram", bufs=1, space="DRAM") as dram,
            tc.tile_pool(name="const", bufs=1) as cpool,
            tc.tile_pool(name="gath", bufs=7) as gpool,
            tc.tile_pool(name="idxp", bufs=6) as ipool,
            tc.tile_pool(name="sbig", bufs=1) as sbig,
            tc.tile_pool(name="work", bufs=2) as wpool,
            tc.tile_pool(name="spool", bufs=3) as spool,
            tc.tile_pool(name="stage", bufs=4) as stpool,
            tc.tile_pool(name="psum", bufs=2, space="PSUM") as ppool,
            tc.tile_pool(name="psum2", bufs=2, space="PSUM") as ppool2,
        ):
            # DRAM working tensors (pool tiles so Tile tracks deps).
            # partials/rs split into NCHUNK window-ranges; each chunk's
            # ReduceScatter(add) is issued as soon as its windows flush.
            tabs = [dram.tile([TABROWS, F1], dt.bfloat16, tag=f"tab{i}", name=f"tab{i}") for i in range(4)]
            FW = [F1, F1, H, F2]
            parts = []
            rss = []
            for i, fw in enumerate(FW):
                pc_, rc_ = [], []
                for q in range(NCHUNK):
                    pc_.append(dram.tile([NC * hq[q] * 128, fw], dt.bfloat16,
                                         tag=f"p{i}_{q}", name=f"p{i}_{q}"))
                    rc_.append(dram.tile([hq[q] * 128, fw], dt.bfloat16,
                                         tag=f"r{i}_{q}", name=f"r{i}_{q}"))
                parts.append(pc_)
                rss.append(rc_)

            # constants
            xT = cpool.tile([F_IN, SL], dt.bfloat16)
            nc.sync.dma_start(out=xT[:], in_=xT_p[:])
            w1f = cpool.tile([F_IN, 256], dt.bfloat16)
            nc.sync.dma_start(out=w1f[:], in_=w1f_p[:])
            w1b = cpool.tile([F1, F1], dt.bfloat16)
            nc.sync.dma_start(out=w1b[:], in_=w1b_p[:])
            iw2 = cpool.tile([H, F2], dt.bfloat16)
            nc.sync.dma_start(out=iw2[:], in_=iw2_p[:])
            rw2 = cpool.tile([H, F2], dt.bfloat16)
            nc.sync.dma_start(out=rw2[:], in_=rw2_p[:])
            w2b = cpool.tile([F2, F2], dt.bfloat16)
            nc.sync.dma_start(out=w2b[:], in_=w2b_p[:])
            b1t = cpool.tile([128, F1], dt.float32)
            nc.sync.dma_start(out=b1t[:], in_=b1_p[:])
            b2t = cpool.tile([128, F2], dt.float32)
            nc.sync.dma_start(out=b2t[:], in_=b2_p[:])
            iota = cpool.tile([128, 128], dt.bfloat16)
            nc.sync.dma_start(out=iota[:], in_=iota_p[:])
            eye = cpool.tile([128, 128], dt.bfloat16)
            nc.sync.dma_start(out=eye[:], in_=eye_p[:])
            zrow = cpool.tile([128, F1], dt.bfloat16)
            nc.vector.memset(zrow[:], 0.0)

            # persistent per-layer state
            rootL1 = sbig.tile([128, WPC, F1], dt.bfloat16, tag="rootL1")
            root2 = sbig.tile([128, WPC, F2], dt.bfloat16, tag="root2")
            dis = sbig.tile([128, WPC], dt.float32, tag="dis")
            nc.sync.dma_start(out=dis[:, :], in_=dis_p[:])

            # zero rows of the tables
            for t in tabs:
                nc.sync.dma_start(out=t[SL:SL + 1, :], in_=zrow[0:1, :])

            def dis_b(j0, nb, width):
                """dis[:, j0:j0+nb] broadcast over `width` features."""
                s = dis[:, j0:j0 + nb]
                return bass.AP(s.tensor, s.offset, s.ap + [[0, width]])

            def row_b(t, nb):
                """[128, Fw] tile broadcast over nb windows (middle dim)."""
                s = t[:, :]
                return bass.AP(s.tensor, s.offset, [s.ap[0], [0, nb], s.ap[1]])

            def col_b(t, nb, width):
                """[128, nb] tile broadcast over width (inner dim)."""
                return bass.AP(t.tensor, t.offset, t.ap + [[0, width]])

            def dram_rows(tile, r0, nrow, fw, fused=None):
                """DRAM rows [r0*128, (r0+nrow)*128) as [128, nrow, fw] AP."""
                fu = fw if fused is None else fused
                s = tile[:]
                return bass.AP(s.tensor, s.offset + r0 * 128 * fw,
                               [[fw, 128], [128 * fw, nrow], [1, fu]])

            def build_S(tg_tile, width):
                """S tile [128, width, 128] = (tgtoff[:, 0:width] == iota)."""
                S = spool.tile([128, GROUPS_PER_CALL, 128], dt.bfloat16, tag="S")
                src = tg_tile[:, 0:width]
                in0 = bass.AP(src.tensor, src.offset, src.ap + [[0, 128]])
                it = iota[:, :]
                in1 = bass.AP(it.tensor, it.offset,
                              [it.ap[0], [0, width], it.ap[1]])
                nc.vector.tensor_tensor(
                    out=S[:, 0:width, :], in0=in0, in1=in1,
                    op=mybir.AluOpType.is_equal)
                return S

            PREFETCH = 3

            def load_idx_tiles(k):
                g0 = k * GROUPS_PER_CALL
                ng = min(GROUPS_PER_CALL, G_total - g0)
                it = ipool.tile([128, GROUPS_PER_CALL * 8], dt.int16, tag="idx")
                nc.sync.dma_start(out=it[:, 0:ng * 8],
                                  in_=idx_p[:, g0 * 8:(g0 + ng) * 8])
                tg = ipool.tile([128, GROUPS_PER_CALL], dt.bfloat16, tag="tg")
                nc.scalar.dma_start(out=tg[:, 0:ng], in_=tgo_p[:, g0:g0 + ng])
                return ng, it, tg

            def do_gather(k, ng, it, table):
                nidx = ng * 128
                gt = gpool.tile([128, GROUPS_PER_CALL, F1], dt.bfloat16, tag="gt")
                nc.gpsimd.dma_gather(
                    gt[:, 0:ng, :], table[:], it[:, 0:ng * 8],
                    num_idxs=nidx, num_idxs_reg=nidx, elem_size=F1,
                    single_packet=False, queue_num=k % 4)
                return gt

            def prop(scope, table, F_used, part_chunks, rs_chunks, loop_iters):
                """loop_iters(q) -> list of thunks; each emits one batched
                post-RS loop iteration for chunk q. They are interleaved
                into subsequent calls' emission so they overlap the prop."""
                bank = [None]
                pending = []   # (emit_at_call, thunk)

                def flushq(c_w, j_w):
                    stq = stpool.tile([128, 4, 128], dt.bfloat16,
                                      tag="st", name="stq")
                    src4 = bank[0][:].rearrange(
                        "p (b f) -> p b f", b=4)[:, :, 0:F_used]
                    nc.scalar.activation(stq[:, :, 0:F_used], src4,
                                         mybir.ActivationFunctionType.Copy)
                    c0 = c_w - 3
                    q = next(qq for qq in range(NCHUNK)
                             if bounds[qq] <= j_w < bounds[qq + 1])
                    jh = j_w - bounds[q]
                    half = part_chunks[q]
                    dst = bass.AP(
                        half.tensor,
                        half.offset + (c0 * hq[q] * 128 + jh * 128) * F_used,
                        [[F_used, 128], [hq[q] * 128 * F_used, 4],
                         [1, F_used]])
                    nc.scalar.dma_start(out=dst, in_=stq[:, :, 0:F_used])
                    return q

                with nc.named_scope(scope):
                    loaded = {}
                    for k in range(min(PREFETCH, NCALLS)):
                        loaded[k] = load_idx_tiles(k)
                    for k in range(NCALLS):
                        if k + PREFETCH < NCALLS:
                            loaded[k + PREFETCH] = load_idx_tiles(k + PREFETCH)
                        ng, it, tg = loaded.pop(k)
                        gt = do_gather(k, ng, it, table)
                        S = build_S(tg, ng)
                        for i in range(ng):
                            g = k * GROUPS_PER_CALL + i
                            w, first, last = sched[g]
                            c_w, j_w = w // WPC, w % WPC
                            qb = c_w % 4
                            if first and qb == 0:
                                bank[0] = ppool.tile([128, 512], dt.float32,
                                                     tag="ps", name="psbank")
                            nc.tensor.matmul(
                                bank[0][:, qb * 128:qb * 128 + F_used],
                                S[:, i, :], gt[:, i, 0:F_used],
                                start=first, stop=last)
                            if last and qb == 3:
                                q = flushq(c_w, j_w)
                                if c_w == NC - 1 and j_w == bounds[q + 1] - 1:
                                    nc.gpsimd.collective_compute(
                                        "ReduceScatter", mybir.AluOpType.add,
                                        replica_groups=[list(range(NC))],
                                        ins=[part_chunks[q][:]],
                                        outs=[rs_chunks[q][:]])
                                    for ii, th in enumerate(loop_iters(q)):
                                        pending.append(
                                            (k + DELAY_CALLS + ii // 2, th))
                        while pending and pending[0][0] <= k:
                            pending.pop(0)[1]()
                    for _, th in pending:
                        th()

            # ---------------- layer 1 setup: root1 + t0 table (2-window batches)
            j0 = 0
            while j0 < WPC:
                nb = min(2, WPC - j0)
                psA = ppool2.tile([128, 512], dt.float32, tag="mm")
                for v in range(nb):
                    nc.tensor.matmul(psA[:, v * 256:(v + 1) * 256],
                                     xT[:, (j0 + v) * 128:(j0 + v + 1) * 128],
                                     w1f[:], start=True, stop=True)
                ps = psA[:]
                root_src = bass.AP(ps.tensor, ps.offset + 128,
                                   [ps.ap[0], [256, nb], [1, 128]])
                nc.scalar.activation(rootL1[:, j0:j0 + nb, :], root_src,
                                     mybir.ActivationFunctionType.Copy)
                h_src = bass.AP(ps.tensor, ps.offset,
                                [ps.ap[0], [256, nb], [1, 128]])
                hs0 = stpool.tile([128, 2, F1], dt.bfloat16, tag="hs0")
                nc.vector.tensor_tensor(out=hs0[:, 0:nb, :], in0=h_src,
                                        in1=dis_b(j0, nb, F1),
                                        op=mybir.AluOpType.mult)
                nc.scalar.dma_start(out=dram_rows(tabs[0], j0, nb, F1),
                                    in_=hs0[:, 0:nb, :])
                j0 += nb

            # ---------------- post-prop loop bodies (4-window batches)

            def loop1_iters(q, src_rs, dst_tab):
                """relu(agg*dis + root + b) -> o; table = dis * (o @ w1blk)."""
                thunks = []
                j0 = bounds[q]
                while j0 < bounds[q + 1]:
                    nb = min(4, bounds[q + 1] - j0)
                    jr = j0 - bounds[q]

                    def th(j0=j0, nb=nb, jr=jr):
                        chb = wpool.tile([128, 4, F1], dt.bfloat16, tag="chb")
                        nc.sync.dma_start(
                            out=chb[:, 0:nb, :],
                            in_=dram_rows(src_rs, jr, nb, F1))
                        ch = wpool.tile([128, 4, F1], dt.float32, tag="ch")
                        nc.vector.tensor_tensor(
                            out=ch[:, 0:nb, :], in0=chb[:, 0:nb, :],
                            in1=dis_b(j0, nb, F1), op=mybir.AluOpType.mult)
                        nc.vector.tensor_tensor(
                            out=ch[:, 0:nb, :], in0=ch[:, 0:nb, :],
                            in1=rootL1[:, j0:j0 + nb, :],
                            op=mybir.AluOpType.add)
                        nc.vector.tensor_tensor(
                            out=ch[:, 0:nb, :], in0=ch[:, 0:nb, :],
                            in1=row_b(b1t, nb), op=mybir.AluOpType.add)
                        o0 = wpool.tile([128, 4, F1], dt.bfloat16, tag="o0")
                        nc.scalar.activation(o0[:, 0:nb, :], ch[:, 0:nb, :],
                                             mybir.ActivationFunctionType.Relu)
                        pT = ppool2.tile([128, 512], dt.bfloat16, tag="tp")
                        for v in range(nb):
                            nc.tensor.transpose(pT[:, v * 128:(v + 1) * 128],
                                                o0[:, v, :], eye[:])
                        o0T = wpool.tile([128, 512], dt.bfloat16, tag="o0T")
                        nc.scalar.activation(o0T[:, 0:nb * 128],
                                             pT[:, 0:nb * 128],
                                             mybir.ActivationFunctionType.Copy)
                        pB = ppool2.tile([128, 512], dt.float32, tag="mm")
                        for v in range(nb):
                            nc.tensor.matmul(pB[:, v * 128:(v + 1) * 128],
                                             o0T[:, v * 128:(v + 1) * 128],
                                             w1b[:], start=True, stop=True)
                        t1r = stpool.tile([128, 4, F1], dt.bfloat16, tag="t1r")
                        pb = pB[:]
                        pb3 = bass.AP(pb.tensor, pb.offset,
                                      [pb.ap[0], [128, nb], [1, F1]])
                        nc.vector.tensor_tensor(
                            out=t1r[:, 0:nb, :], in0=pb3,
                            in1=dis_b(j0, nb, F1), op=mybir.AluOpType.mult)
                        nc.scalar.dma_start(
                            out=dram_rows(dst_tab, j0, nb, F1),
                            in_=t1r[:, 0:nb, :])
                    thunks.append(th)
                    j0 += nb
                return thunks

            def loop2_iters(q):
                """h = mean-stack(relu(...)); tab2 rows = dis*h; root2."""
                thunks = []
                j0 = bounds[q]
                while j0 < bounds[q + 1]:
                    nb = min(4, bounds[q + 1] - j0)
                    jr = j0 - bounds[q]

                    def th(j0=j0, nb=nb, jr=jr):
                        chb = wpool.tile([128, 4, F1], dt.bfloat16, tag="chb")
                        nc.sync.dma_start(
                            out=chb[:, 0:nb, :],
                            in_=dram_rows(rss[1][q], jr, nb, F1))
                        ch = wpool.tile([128, 4, F1], dt.float32, tag="ch")
                        nc.vector.tensor_tensor(
                            out=ch[:, 0:nb, :], in0=chb[:, 0:nb, :],
                            in1=dis_b(j0, nb, F1), op=mybir.AluOpType.mult)
                        nc.vector.tensor_tensor(
                            out=ch[:, 0:nb, :], in0=ch[:, 0:nb, :],
                            in1=rootL1[:, j0:j0 + nb, :],
                            op=mybir.AluOpType.add)
                        nc.vector.tensor_tensor(
                            out=ch[:, 0:nb, :], in0=ch[:, 0:nb, :],
                            in1=row_b(b1t, nb), op=mybir.AluOpType.add)
                        o1 = wpool.tile([128, 4, F1], dt.float32, tag="o1")
                        nc.scalar.activation(o1[:, 0:nb, :], ch[:, 0:nb, :],
                                             mybir.ActivationFunctionType.Relu)
                        hh = wpool.tile([128, 4, H], dt.bfloat16, tag="hh")
                        o1a = o1[:, 0:nb, 0:H]
                        ob = o1[:, 0:nb, :]
                        o1b = bass.AP(ob.tensor, ob.offset + H, o1a.ap)
                        nc.vector.tensor_tensor(out=hh[:, 0:nb, :], in0=o1a,
                                                in1=o1b,
                                                op=mybir.AluOpType.add)
                        nc.vector.tensor_scalar_mul(hh[:, 0:nb, :],
                                                    hh[:, 0:nb, :], 0.5)
                        hdis = stpool.tile([128, 4, H], dt.bfloat16,
                                           tag="hdis")
                        nc.vector.tensor_tensor(
                            out=hdis[:, 0:nb, :], in0=hh[:, 0:nb, :],
                            in1=dis_b(j0, nb, H), op=mybir.AluOpType.mult)
                        nc.scalar.dma_start(
                            out=dram_rows(tabs[2], j0, nb, F1, fused=H),
                            in_=hdis[:, 0:nb, :])
                        # root2 = h @ rw2 (fused stacks)
                        pT = ppool2.tile([128, 512], dt.bfloat16, tag="tp")
                        for v in range(nb):
                            nc.tensor.transpose(
                                pT[0:H, v * 128:(v + 1) * 128],
                                hh[:, v, :], eye[:])
                        hT = wpool.tile([H, 512], dt.bfloat16, tag="hT")
                        nc.scalar.activation(hT[:, 0:nb * 128],
                                             pT[0:H, 0:nb * 128],
                                             mybir.ActivationFunctionType.Copy)
                        pC = ppool2.tile([128, 4 * F2], dt.float32, tag="mm2")
                        for v in range(nb):
                            nc.tensor.matmul(pC[:, v * F2:(v + 1) * F2],
                                             hT[:, v * 128:(v + 1) * 128],
                                             rw2[:], start=True, stop=True)
                        pcs = pC[:]
                        pc3 = bass.AP(pcs.tensor, pcs.offset,
                                      [pcs.ap[0], [F2, nb], [1, F2]])
                        nc.scalar.activation(root2[:, j0:j0 + nb, :], pc3,
                                             mybir.ActivationFunctionType.Copy)
                    thunks.append(th)
                    j0 += nb
                return thunks

            def loop3_iters(q):
                """z=agg*dis; o=relu(z@iw2+root2+b2); tab3 = dis*(o@w2blk)."""
                thunks = []
                j0 = bounds[q]
                while j0 < bounds[q + 1]:
                    nb = min(4, bounds[q + 1] - j0)
                    jr = j0 - bounds[q]

                    def th(j0=j0, nb=nb, jr=jr):
                        chb = wpool.tile([128, 4, H], dt.bfloat16, tag="chb2")
                        nc.sync.dma_start(
                            out=chb[:, 0:nb, :],
                            in_=dram_rows(rss[2][q], jr, nb, H))
                        zb = wpool.tile([128, 4, H], dt.bfloat16, tag="zb")
                        nc.vector.tensor_tensor(
                            out=zb[:, 0:nb, :], in0=chb[:, 0:nb, :],
                            in1=dis_b(j0, nb, H), op=mybir.AluOpType.mult)
                        pT = ppool2.tile([128, 512], dt.bfloat16, tag="tp")
                        for v in range(nb):
                            nc.tensor.transpose(
                                pT[0:H, v * 128:(v + 1) * 128],
                                zb[:, v, :], eye[:])
                        zT = wpool.tile([H, 512], dt.bfloat16, tag="zT")
                        nc.scalar.activation(zT[:, 0:nb * 128],
                                             pT[0:H, 0:nb * 128],
                                             mybir.ActivationFunctionType.Copy)
                        pD = ppool2.tile([128, 4 * F2], dt.float32, tag="mm2")
                        for v in range(nb):
                            nc.tensor.matmul(pD[:, v * F2:(v + 1) * F2],
                                             zT[:, v * 128:(v + 1) * 128],
                                             iw2[:], start=True, stop=True)
                        pd = pD[:]
                        pd3 = bass.AP(pd.tensor, pd.offset,
                                      [pd.ap[0], [F2, nb], [1, F2]])
                        nc.vector.tensor_tensor(out=pd3, in0=pd3,
                                                in1=root2[:, j0:j0 + nb, :],
                                                op=mybir.AluOpType.add)
                        nc.vector.tensor_tensor(out=pd3, in0=pd3,
                                                in1=row_b(b2t, nb),
                                                op=mybir.AluOpType.add)
                        o20 = wpool.tile([128, 4, F2], dt.bfloat16, tag="o20")
                        nc.scalar.activation(o20[:, 0:nb, :], pd3,
                                             mybir.ActivationFunctionType.Relu)
                        pT2 = ppool2.tile([128, 512], dt.bfloat16, tag="tp")
                        for v in range(nb):
                            nc.tensor.transpose(
                                pT2[0:F2, v * 128:(v + 1) * 128],
                                o20[:, v, :], eye[:])
                        oT = wpool.tile([F2, 512], dt.bfloat16, tag="oT")
                        nc.scalar.activation(oT[:, 0:nb * 128],
                                             pT2[0:F2, 0:nb * 128],
                                             mybir.ActivationFunctionType.Copy)
                        pE = ppool2.tile([128, 4 * F2], dt.float32, tag="mm2")
                        for v in range(nb):
                            nc.tensor.matmul(pE[:, v * F2:(v + 1) * F2],
                                             oT[:, v * 128:(v + 1) * 128],
                                             w2b[:], start=True, stop=True)
                        t1r = stpool.tile([128, 4, F2], dt.bfloat16,
                                          tag="t1r2")
                        pe = pE[:]
                        pe3 = bass.AP(pe.tensor, pe.offset,
                                      [pe.ap[0], [F2, nb], [1, F2]])
                        nc.vector.tensor_tensor(
                            out=t1r[:, 0:nb, :], in0=pe3,
                            in1=dis_b(j0, nb, F2), op=mybir.AluOpType.mult)
                        nc.scalar.dma_start(
                            out=dram_rows(tabs[3], j0, nb, F1, fused=F2),
                            in_=t1r[:, 0:nb, :])
                    thunks.append(th)
                    j0 += nb
                return thunks

            def loop4_iters(q):
                """final: relu, mean stacks, log_softmax, store."""
                thunks = []
                j0 = bounds[q]
                while j0 < bounds[q + 1]:
                    nb = min(4, bounds[q + 1] - j0)
                    jr = j0 - bounds[q]

                    def th(j0=j0, nb=nb, jr=jr):
                        chb = wpool.tile([128, 4, F2], dt.bfloat16, tag="chb3")
                        nc.sync.dma_start(
                            out=chb[:, 0:nb, :],
                            in_=dram_rows(rss[3][q], jr, nb, F2))
                        z = wpool.tile([128, 4, F2], dt.float32, tag="z")
                        nc.vector.tensor_tensor(
                            out=z[:, 0:nb, :], in0=chb[:, 0:nb, :],
                            in1=dis_b(j0, nb, F2), op=mybir.AluOpType.mult)
                        nc.vector.tensor_tensor(
                            out=z[:, 0:nb, :], in0=z[:, 0:nb, :],
                            in1=root2[:, j0:j0 + nb, :],
                            op=mybir.AluOpType.add)
                        nc.vector.tensor_tensor(
                            out=z[:, 0:nb, :], in0=z[:, 0:nb, :],
                            in1=row_b(b2t, nb), op=mybir.AluOpType.add)
                        o21 = wpool.tile([128, 4, F2], dt.float32, tag="o21")
                        nc.scalar.activation(o21[:, 0:nb, :], z[:, 0:nb, :],
                                             mybir.ActivationFunctionType.Relu)
                        zm = wpool.tile([128, 4, C], dt.float32, tag="zm")
                        oa = o21[:, 0:nb, 0:C]
                        obase = o21[:, 0:nb, :]
                        obb = bass.AP(obase.tensor, obase.offset + C, oa.ap)
                        nc.vector.tensor_tensor(out=zm[:, 0:nb, :], in0=oa,
                                                in1=obb,
                                                op=mybir.AluOpType.add)
                        nc.vector.tensor_scalar_mul(zm[:, 0:nb, :],
                                                    zm[:, 0:nb, :], 0.5)
                        mx = wpool.tile([128, 4], dt.float32, tag="mx")
                        nc.vector.tensor_reduce(mx[:, 0:nb], zm[:, 0:nb, :],
                                                mybir.AxisListType.X,
                                                mybir.AluOpType.max)
                        tt = wpool.tile([128, 4, C], dt.float32, tag="tt")
                        nc.vector.tensor_tensor(
                            out=tt[:, 0:nb, :], in0=zm[:, 0:nb, :],
                            in1=col_b(mx[:, 0:nb], nb, C),
                            op=mybir.AluOpType.subtract)
                        ex = wpool.tile([128, 4, C], dt.float32, tag="ex")
                        nc.scalar.activation(ex[:, 0:nb, :], tt[:, 0:nb, :],
                                             mybir.ActivationFunctionType.Exp)
                        sm = wpool.tile([128, 4], dt.float32, tag="sm")
                        nc.vector.tensor_reduce(sm[:, 0:nb], ex[:, 0:nb, :],
                                                mybir.AxisListType.X,
                                                mybir.AluOpType.add)
                        ls = wpool.tile([128, 4], dt.float32, tag="ls")
                        nc.scalar.activation(ls[:, 0:nb], sm[:, 0:nb],
                                             mybir.ActivationFunctionType.Ln)
                        res = wpool.tile([128, 4, C], dt.float32, tag="res")
                        nc.vector.tensor_tensor(
                            out=res[:, 0:nb, :], in0=tt[:, 0:nb, :],
                            in1=col_b(ls[:, 0:nb], nb, C),
                            op=mybir.AluOpType.subtract)
                        nc.scalar.dma_start(
                            out=dram_rows(out_p, j0, nb, C),
                            in_=res[:, 0:nb, :])
                    thunks.append(th)
                    j0 += nb
                return thunks

            # ---------------- the four propagation rounds
            prop("prop1", tabs[0], F1, parts[0], rss[0],
                 lambda q: loop1_iters(q, rss[0][q], tabs[1]))
            prop("prop2", tabs[1], F1, parts[1], rss[1], loop2_iters)
            prop("prop3", tabs[2], H, parts[2], rss[2], loop3_iters)
            prop("prop4", tabs[3], F2, parts[3], rss[3], loop4_iters)

    nc.finalize()
    return nc


# ------------------------------------------------------------------ runner

last_exec_time_ns = None
last_scope_times = None


def kernel(x, edge_index, init_w1, w1, root_w1, b1, init_w2, w2, root_w2, b2):
    global last_exec_time_ns, last_scope_times
    from concourse.bass_utils import run_bass_kernel_spmd

    x = np.asarray(x, np.float32)
    P, idx_dram, tgo_dram, xT, G_total, sched, dis_core = _prep(x, edge_index)

    iw1 = np.asarray(init_w1, np.float32)
    rw1 = np.asarray(root_w1, np.float32)
    w1a = np.asarray(w1, np.float32)
    iw2a = np.asarray(init_w2, np.float32)
    rw2a = np.asarray(root_w2, np.float32)
    w2a = np.asarray(w2, np.float32)
    b1a = np.asarray(b1, np.float32)
    b2a = np.asarray(b2, np.float32)

    w1f = np.concatenate([iw1[0], iw1[1], rw1[0], rw1[1]], axis=1)   # [100,256]
    w1blk = np.zeros((F1, F1), np.float32)
    w1blk[0:H, 0:H] = w1a[0]
    w1blk[H:F1, H:F1] = w1a[1]
    iw2f = np.concatenate([iw2a[0], iw2a[1]], axis=1)                # [64,36]
    rw2f = np.concatenate([rw2a[0], rw2a[1]], axis=1)                # [64,36]
    w2blk = np.zeros((F2, F2), np.float32)
    w2blk[0:C, 0:C] = w2a[0]
    w2blk[C:F2, C:F2] = w2a[1]
    b1row = np.concatenate([b1a[0, 0], b1a[1, 0]])                   # [128]
    b2row = np.concatenate([b2a[0, 0], b2a[1, 0]])                   # [36]
    b1t = np.tile(b1row[None, :], (128, 1)).astype(np.float32)
    b2t = np.tile(b2row[None, :], (128, 1)).astype(np.float32)
    iota = np.tile(np.arange(128, dtype=np.float32)[None, :],
                   (128, 1)).astype(BF16)
    eye = np.eye(128, dtype=np.float32).astype(BF16)

    print(f"[kernel] G_total={G_total} "
          f"calls/prop={(G_total + GROUPS_PER_CALL - 1) // GROUPS_PER_CALL}")
    nc = _build(G_total, sched, None)

    in_maps = []
    for c in range(NC):
        in_maps.append({
            "xT": np.ascontiguousarray(xT[c]),
            "idx": np.ascontiguousarray(idx_dram[c]),
            "tgo": np.ascontiguousarray(tgo_dram[c]),
            "w1f": w1f.astype(BF16),
            "w1b": w1blk.astype(BF16),
            "iw2": iw2f.astype(BF16),
            "rw2": rw2f.astype(BF16),
            "w2b": w2blk.astype(BF16),
            "b1t": b1t,
            "b2t": b2t,
            "iota": iota,
            "eye": eye,
            "dis": np.ascontiguousarray(dis_core[c]),
        })

    trace = _install_ntff_hook() and os.environ.get("KERNEL_NO_TRACE") != "1"
    try:
        res = run_bass_kernel_spmd(nc, in_maps, core_ids=list(range(NC)),
                                   trace=trace)
    except Exception:
        if not trace:
            raise
        res = run_bass_kernel_spmd(nc, in_maps, core_ids=list(range(NC)),
                                   trace=False)
    last_exec_time_ns = res.exec_time_ns
    last_scope_times = res.per_core_scope_times

    full = np.concatenate([np.asarray(res.results[c]["out"], np.float32)
                           for c in range(NC)], axis=0)       # [NPAD, C]
    return full[P]                                            # [N, C]
